# revision 7
# baseline (speedup 1.0000x reference)
"""Trainium2 Bass kernel for Tacotron-style location attention.

Computes, for B=128, T=2000, D_Q=1024, D_MEM=512, D_ATT=128:
    pq      = hidden @ Wq.T                         # (B, A)
    E[b,t]  = v . tanh(pq[b] + pm[b,t])             # (B, T)
    W       = softmax(where(mask, -inf, E), axis=1) # (B, T)
    ctx     = einsum('bt,btd->bd', W, memory)       # (B, D_MEM)
returns (ctx, W).

Sharding: data-parallel over batch across 8 NeuronCores (16 rows/core);
Wq and v replicated. Per-core layout strategy:
  - T is tiled into 16 chunks of 125 rows; pm/memory slabs are DMA'd per
    batch row as (125p, 16c, feat) so each chunk sits on partitions.
  - energies: PE-transpose pm chunks to (A=128p, t), ScalarE computes
    tanh(x + pq_b) (bias add fused), PE contracts with v into PSUM rows
    (one partition per batch row) -> E lives as (16p, T) for a trivially
    vectorized masked softmax along the free axis.
  - context: PE-transpose W chunks to (125t, 16b) columns, then 16
    accumulating matmuls per row: ctx[b] += W_chunk.T @ mem_chunk.
"""

import numpy as np

import concourse.bass as bass
from concourse import bacc
import concourse.mybir as mybir
import concourse.tile as tile
from concourse.bass_utils import run_bass_kernel_spmd
from concourse.masks import make_identity

N_CORES = 8
B, T, DQ, DM, A = 128, 2000, 1024, 512, 128
BL = B // N_CORES          # 16 batch rows per core
TC = 125                   # t-chunk size (T = 16 * 125, no tail)
NCH = T // TC              # 16 chunks
NG = 4                     # chunk groups of 4 -> 500-wide tiles
GW = NG * TC               # 500
NEG_BIG = -1.0e38          # masked fill; exp underflows to exactly 0.0

F32 = mybir.dt.float32
U8 = mybir.dt.uint8


def build_nc() -> bass.Bass:
    nc = bacc.Bacc("TRN2", target_bir_lowering=False, debug=False, num_devices=N_CORES)

    hid = nc.dram_tensor("hid", (BL, DQ), F32, kind="ExternalInput")
    mem = nc.dram_tensor("mem", (BL, T, DM), F32, kind="ExternalInput")
    pm = nc.dram_tensor("pm", (BL, T, A), F32, kind="ExternalInput")
    msk = nc.dram_tensor("msk", (BL, T), U8, kind="ExternalInput")
    wq = nc.dram_tensor("wq", (A, DQ), F32, kind="ExternalInput")
    vv = nc.dram_tensor("v", (1, A), F32, kind="ExternalInput")
    ctx_out = nc.dram_tensor("ctx_out", (BL, DM), F32, kind="ExternalOutput")
    w_out = nc.dram_tensor("w_out", (BL, T), F32, kind="ExternalOutput")

    with tile.TileContext(nc) as tc:
        with (
            tc.tile_pool(name="singles", bufs=1) as singles,
            tc.tile_pool(name="pmp", bufs=3) as pmp,
            tc.tile_pool(name="memp", bufs=3) as memp,
            tc.tile_pool(name="tanhp", bufs=3) as tanhp,
            tc.tile_pool(name="ps_tr", bufs=3, space="PSUM") as ps_tr,
            tc.tile_pool(name="ps_e", bufs=3, space="PSUM") as ps_e,
            tc.tile_pool(name="ps_ctx", bufs=2, space="PSUM") as ps_ctx,
        ):
            # ---- constants / small loads ----
            idt = singles.tile([128, 128], F32)
            make_identity(nc, idt)
            v_col = singles.tile([A, 1], F32)
            nc.sync.dma_start(out=v_col, in_=vv[:, :].rearrange("o a -> a o"))
            wq_sb = singles.tile([A, DQ], F32)
            nc.sync.dma_start(out=wq_sb, in_=wq[:, :])
            hid_sb = singles.tile([BL, DQ], F32)
            nc.sync.dma_start(out=hid_sb, in_=hid[:, :])
            msk_sb = singles.tile([BL, T], U8)
            nc.sync.dma_start(out=msk_sb, in_=msk[:, :])

            # ---- pq[a,b] = sum_d Wq[a,d] * hid[b,d] (PSUM accum over 8 k-chunks)
            wqT = singles.tile([128, DQ], F32)   # (d_local, a) per k-chunk
            hidT = singles.tile([128, 8, BL], F32)  # (d_local, k, b)
            for k in range(8):
                t1 = ps_tr.tile([128, GW], F32, tag="tr")
                nc.tensor.transpose(t1[:, :128], wq_sb[:, bass.ts(k, 128)], idt)
                nc.vector.tensor_copy(wqT[:, bass.ts(k, 128)], t1[:, :128])
                t2 = ps_tr.tile([128, GW], F32, tag="tr")
                nc.tensor.transpose(
                    t2[:, :BL], hid_sb[:, bass.ts(k, 128)], idt[:BL, :BL]
                )
                nc.vector.tensor_copy(hidT[:, k, :], t2[:, :BL])
            pq_ps = ps_tr.tile([128, GW], F32, tag="tr")
            for k in range(8):
                nc.tensor.matmul(
                    pq_ps[:, :BL],
                    wqT[:, bass.ts(k, 128)],
                    hidT[:, k, :],
                    start=(k == 0),
                    stop=(k == 7),
                    skip_group_check=True,
                )
            pq_sb = singles.tile([128, BL], F32)
            nc.vector.tensor_copy(pq_sb, pq_ps[:, :BL])

            # masked fill values: 0.0 where keep, -1e38 where masked
            maskneg = singles.tile([BL, T], F32)
            nc.vector.tensor_scalar_mul(maskneg, msk_sb, NEG_BIG)

            # ---- phase 1: energies as columns e_t[t_local, c, b], then
            # per-chunk PE transpose into row layout e_sb (16p, 2000)
            e_t = singles.tile([TC, NCH, BL], F32)
            for b in range(BL):
                pm_b = pmp.tile([TC, NCH, A], F32)
                nc.sync.dma_start(
                    out=pm_b, in_=pm[b, :, :].rearrange("(c p) a -> p c a", p=TC)
                )
                e_col = ps_e.tile([TC, NCH], F32, tag="e")
                for g in range(NG):
                    trp = ps_tr.tile([128, GW], F32, tag="tr")
                    for j in range(NG):
                        cc = g * NG + j
                        nc.tensor.transpose(
                            trp[:, bass.ts(j, TC)], pm_b[:, cc, :], idt[:TC, :TC]
                        )
                    th = tanhp.tile([128, GW], F32)
                    nc.scalar.activation(
                        th,
                        trp,
                        mybir.ActivationFunctionType.Tanh,
                        bias=pq_sb[:, b : b + 1],
                    )
                    for j in range(NG):
                        cc = g * NG + j
                        nc.tensor.matmul(
                            e_col[:, cc : cc + 1],
                            th[:, bass.ts(j, TC)],
                            v_col,
                            start=True,
                            stop=True,
                            skip_group_check=True,
                        )
                nc.vector.tensor_copy(e_t[:, :, b], e_col)
            e_sb = singles.tile([BL, T], F32)
            for c in range(NCH):
                te = ps_tr.tile([128, GW], F32, tag="tr")
                nc.tensor.transpose(
                    te[:BL, :TC], e_t[:, c, :], idt[:TC, :TC]
                )
                nc.vector.tensor_copy(e_sb[:, bass.ts(c, TC)], te[:BL, :TC])

            # ---- masked softmax over T (free axis), rows = batch
            nc.vector.tensor_add(e_sb, e_sb, maskneg)
            negmax = singles.tile([BL, 1], F32)
            nc.vector.reduce_max(
                negmax, e_sb, axis=mybir.AxisListType.X, negate=True
            )
            w_sb = singles.tile([BL, T], F32)
            rowsum = singles.tile([BL, 1], F32)
            nc.scalar.activation(
                w_sb,
                e_sb,
                mybir.ActivationFunctionType.Exp,
                bias=negmax,
                accum_out=rowsum,
            )
            rinv = singles.tile([BL, 1], F32)
            nc.vector.reciprocal(rinv, rowsum)
            nc.vector.tensor_scalar_mul(w_sb, w_sb, rinv)
            nc.sync.dma_start(out=w_out[:, :], in_=w_sb)

            # ---- transpose W chunks into columns: wt_all[:, c*16+b] = W[b, c*125:+125]
            wt_all = singles.tile([TC, NCH * BL], F32)
            for c in range(NCH):
                tw = ps_tr.tile([128, GW], F32, tag="tr")
                nc.tensor.transpose(
                    tw[:TC, :BL], w_sb[:, bass.ts(c, TC)], idt[:BL, :BL]
                )
                nc.vector.tensor_copy(wt_all[:, bass.ts(c, BL)], tw[:TC, :BL])

            # ---- phase 2: ctx columns ctx_t[d_local, q, b] via PSUM accum over
            # chunks, then per-q PE transpose into ctx_sb (16p, 512)
            NQ = DM // 128  # 4 d-quarters
            ctx_t = singles.tile([128, NQ, BL], F32)
            for b in range(BL):
                mem_b = memp.tile([TC, NCH, DM], F32)
                nc.sync.dma_start(
                    out=mem_b, in_=mem[b, :, :].rearrange("(c p) d -> p c d", p=TC)
                )
                ctx_col = ps_ctx.tile([128, NQ], F32, tag="ctx")
                for q in range(NQ):
                    for c in range(NCH):
                        nc.tensor.matmul(
                            ctx_col[:, q : q + 1],
                            mem_b[:, c, bass.ts(q, 128)],
                            wt_all[:, c * BL + b : c * BL + b + 1],
                            start=(c == 0),
                            stop=(c == NCH - 1),
                            skip_group_check=True,
                        )
                nc.vector.tensor_copy(ctx_t[:, :, b], ctx_col)
            ctx_sb = singles.tile([BL, DM], F32)
            for q in range(NQ):
                tq = ps_tr.tile([128, GW], F32, tag="tr")
                nc.tensor.transpose(tq[:BL, :128], ctx_t[:, q, :], idt)
                nc.vector.tensor_copy(ctx_sb[:, bass.ts(q, 128)], tq[:BL, :128])
            nc.sync.dma_start(out=ctx_out[:, :], in_=ctx_sb)

    nc.finalize()
    return nc


_NC_CACHE: list = []


def _get_nc() -> bass.Bass:
    if not _NC_CACHE:
        _NC_CACHE.append(build_nc())
    return _NC_CACHE[0]


def make_in_maps(inputs: dict) -> list:
    ahs = np.ascontiguousarray(np.asarray(inputs["attention_hidden_state"], np.float32))
    memory = np.asarray(inputs["memory"], np.float32)
    pm = np.asarray(inputs["processed_memory"], np.float32)
    mask = np.asarray(inputs["mask"]).astype(np.uint8)
    wq = np.ascontiguousarray(np.asarray(inputs["Wq"], np.float32))
    v = np.ascontiguousarray(np.asarray(inputs["v"], np.float32))
    in_maps = []
    for i in range(N_CORES):
        sl = slice(i * BL, (i + 1) * BL)
        in_maps.append(
            {
                "hid": np.ascontiguousarray(ahs[sl]),
                "mem": np.ascontiguousarray(memory[sl]),
                "pm": np.ascontiguousarray(pm[sl]),
                "msk": np.ascontiguousarray(mask[sl]),
                "wq": wq,
                "v": v,
            }
        )
    return in_maps


def run(inputs: dict, trace: bool = False):
    """Run on 8 cores; returns ((ctx, weights), BassKernelResults)."""
    nc = _get_nc()
    res = run_bass_kernel_spmd(
        nc, make_in_maps(inputs), list(range(N_CORES)), trace=trace
    )
    ctx = np.concatenate([r["ctx_out"] for r in res.results], axis=0)
    w = np.concatenate([r["w_out"] for r in res.results], axis=0)
    return (ctx, w), res


def kernel(**inputs):
    (ctx, w), _ = run(inputs, trace=False)
    return ctx, w
